# revision 7
# baseline (speedup 1.0000x reference)
"""Dilated attention (B=4,S=4096,D=768,H=12,DIL=8) on 8 TRN2 NeuronCores.

Sharding: batch x seq-half data-parallel -> core c handles batch c//2,
sequence half c%2 (2048 query tokens). The DIL-strided K/V positions
(512 per batch) are position-independent, so each core computes K/V for
its batch's 512 dilated positions locally (replicated within the
batch's core pair).

Host prep (layout only): x chunks and the dilated x are transposed to
[D, T] so the d-dim lands on SBUF partitions (PE contraction dim);
weights are passed as W.T ([in, out]). All matmuls run as float32r
(TF32-like, full-rate on PE at N>=256, ~1.5e-4 rel err measured).

On-chip layouts (d-on-partition tensors are [128, 6, T] with
d = ko*128 + p):
  QT [d, t], KT [d, j]: per-head slices are [64, .] at partition base
  (h%2)*64 of d-tile h//2 -- adjacent heads row-pack the PE array.
  scores^T [j, t] per head; softmax denominator comes from a ones
  column appended to V (ctx matmul computes [V|1]^T @ expS -> [ctx;
  denom]). No max-subtraction: scores here are O(1) (|s|<~3), exp is
  safe in f32.
"""
import sys
sys.path.insert(0, "/opt/trn_rl_repo")
import numpy as np

import concourse.bacc as bacc
import concourse.tile as tile
from concourse import mybir
from concourse.bass_utils import run_bass_kernel_spmd

B, S, D, H, DIL = 4, 4096, 768, 12, 8
HD = D // H            # 64
SD = S // DIL          # 512 dilated K/V positions
NCORE = 8
TOK = B * S // NCORE   # 2048 query tokens per core
TCH = 512              # chunk of query tokens processed at once
NCH = TOK // TCH       # 4
NKT = D // 128         # 6 contraction tiles
NJT = SD // 128        # 4 j tiles
F32 = mybir.dt.float32
F32R = mybir.dt.float32r
SCALE = 1.0 / float(np.sqrt(HD))
EXP = mybir.ActivationFunctionType.Exp
ADD = mybir.AluOpType.add
MULT = mybir.AluOpType.mult

_CACHE = {}


def _head_slice(t, h, cols):
    """[64, ...] slice of a [128, NKT, T] d-on-partition tensor for head h."""
    base = (h % 2) * HD
    return t[base:base + HD, h // 2, cols]


def _build():
    nc = bacc.Bacc("TRN2", target_bir_lowering=False, debug=False,
                   num_devices=NCORE)

    xT_d = nc.dram_tensor("xT", [D, TOK], F32, kind="ExternalInput")
    xdT_d = nc.dram_tensor("xdT", [D, SD], F32, kind="ExternalInput")
    w_d = {n: nc.dram_tensor(n, [D, D], F32, kind="ExternalInput")
           for n in ("wqT", "wkT", "wvT", "woT")}
    b_d = {n: nc.dram_tensor(n, [D], F32, kind="ExternalInput")
           for n in ("bq", "bk", "bv", "bo")}
    out_d = nc.dram_tensor("out", [TOK, D], F32, kind="ExternalOutput")

    from contextlib import ExitStack
    with tile.TileContext(nc) as tc, ExitStack() as es:
        cpool = es.enter_context(tc.tile_pool(name="const", bufs=1))
        xpool = es.enter_context(tc.tile_pool(name="xin", bufs=1))
        qpool = es.enter_context(tc.tile_pool(name="qt", bufs=2))
        epool = es.enter_context(tc.tile_pool(name="exps", bufs=2))
        ctpool = es.enter_context(tc.tile_pool(name="ctxt", bufs=2))
        opool = es.enter_context(tc.tile_pool(name="outs", bufs=2))
        spool = es.enter_context(tc.tile_pool(name="scal", bufs=2))
        rpool = es.enter_context(tc.tile_pool(name="rbc", bufs=2))
        mmp = es.enter_context(tc.tile_pool(name="mm", bufs=2, space="PSUM"))
        scp = es.enter_context(tc.tile_pool(name="sc", bufs=4, space="PSUM"))
        cxp = es.enter_context(tc.tile_pool(name="cx", bufs=2, space="PSUM"))

        # ---- constants: weights (cast to f32r via gpsimd DMA), biases ----
        w_sb = {}
        for n in ("wqT", "wkT", "wvT", "woT"):
            t = cpool.tile([128, NKT, D], F32R, name=n)
            nc.gpsimd.dma_start(t[:], w_d[n].rearrange("(ko p) o -> p ko o", p=128))
            w_sb[n] = t
        bq_sb = cpool.tile([128, NKT], F32, name="bq")
        nc.sync.dma_start(bq_sb[:], b_d["bq"].rearrange("(ko p) -> p ko", p=128))
        bk_sb = cpool.tile([128, NKT], F32, name="bk")
        nc.sync.dma_start(bk_sb[:], b_d["bk"].rearrange("(ko p) -> p ko", p=128))
        bv_sb = cpool.tile([128, D], F32, name="bv")
        nc.sync.dma_start(bv_sb[:], b_d["bv"][None, :].to_broadcast((128, D)))
        bo_sb = cpool.tile([128, D], F32, name="bo")
        nc.sync.dma_start(bo_sb[:], b_d["bo"][None, :].to_broadcast((128, D)))

        # dilated x (for K/V projections), d on partitions
        xdT_sb = cpool.tile([128, NKT, SD], F32R, name="xdT")
        nc.gpsimd.dma_start(xdT_sb[:], xdT_d.rearrange("(ko p) j -> p ko j", p=128))

        # ---- K^T [d, j] and V [j, head, 64|1] (ones col for denominator) ----
        kT_sb = cpool.tile([128, NKT, SD], F32R, name="kT")
        for m in range(NKT):
            ps = mmp.tile([128, 512], F32, name="mmps")
            for kt in range(NKT):
                nc.tensor.matmul(ps[:], w_sb["wkT"][:, kt, m * 128:(m + 1) * 128],
                                 xdT_sb[:, kt, :], start=(kt == 0),
                                 stop=(kt == NKT - 1))
            nc.vector.tensor_tensor(kT_sb[:, m, :], ps[:],
                                    bk_sb[:, m, None].to_broadcast((128, SD)), ADD)

        v_sb = cpool.tile([128, NJT, H, HD + 1], F32R, name="v")
        nc.vector.memset(v_sb[:, :, :, HD:].bitcast(F32), 1.0)
        for jt in range(NJT):
            for nh0, nh1 in ((0, 8), (8, 12)):
                ncols = (nh1 - nh0) * HD
                ps = mmp.tile([128, 512], F32, name="mmps")
                for kt in range(NKT):
                    nc.tensor.matmul(ps[:, :ncols],
                                     xdT_sb[:, kt, jt * 128:(jt + 1) * 128],
                                     w_sb["wvT"][:, kt, nh0 * HD:nh1 * HD],
                                     start=(kt == 0), stop=(kt == NKT - 1))
                nc.vector.tensor_tensor(
                    v_sb[:, jt, nh0:nh1, :HD],
                    ps[:, :ncols].rearrange("p (h e) -> p h e", e=HD),
                    bv_sb[:, nh0 * HD:nh1 * HD]
                    .rearrange("p (h e) -> p h e", e=HD),
                    ADD)

        # ---- per 512-token chunk: Q^T proj, attention, out proj ----
        for ch in range(NCH):
            tsl = slice(ch * TCH, (ch + 1) * TCH)

            xT_sb = xpool.tile([128, NKT, TCH], F32R, name="xT")
            nc.gpsimd.dma_start(
                xT_sb[:], xT_d.rearrange("(ko p) t -> p ko t", p=128)[:, :, tsl])

            qT_sb = qpool.tile([128, NKT, TCH], F32R, name="qT")
            for m in range(NKT):
                ps = mmp.tile([128, 512], F32, name="mmps")
                for kt in range(NKT):
                    nc.tensor.matmul(ps[:], w_sb["wqT"][:, kt, m * 128:(m + 1) * 128],
                                     xT_sb[:, kt, :], start=(kt == 0),
                                     stop=(kt == NKT - 1))
                nc.vector.tensor_tensor(qT_sb[:, m, :], ps[:],
                                        bq_sb[:, m, None].to_broadcast((128, TCH)),
                                        ADD)

            # attention per head; scores^T [j, t], exp, ctx^T = [V|1]^T @ expS
            ctxT_sb = ctpool.tile([128, NKT, TCH], F32R, name="ctxT")
            for h in range(H):
                sps = [scp.tile([128, TCH], F32, name="scps") for _ in range(NJT)]
                exp_sb = epool.tile([128, NJT, TCH], F32R, name="expS")
                for jt in range(NJT):
                    nc.tensor.matmul(sps[jt][:],
                                     _head_slice(kT_sb, h,
                                                 slice(jt * 128, (jt + 1) * 128)),
                                     _head_slice(qT_sb, h, slice(None)),
                                     start=True, stop=True)
                    nc.scalar.activation(exp_sb[:, jt, :], sps[jt][:], EXP,
                                         scale=SCALE)
                cps = cxp.tile([128, TCH], F32, name="cxps")
                for jt in range(NJT):
                    nc.tensor.matmul(cps[:HD + 1, :], v_sb[:, jt, h, :],
                                     exp_sb[:, jt, :], start=(jt == 0),
                                     stop=(jt == NJT - 1))
                rden = spool.tile([1, TCH], F32, name="rden")
                nc.vector.reciprocal(rden[:], cps[HD:HD + 1, :])
                rbc = rpool.tile([HD, TCH], F32, name="rbc")
                nc.gpsimd.partition_broadcast(rbc[:], rden[:])
                nc.vector.tensor_tensor(_head_slice(ctxT_sb, h, slice(None)),
                                        cps[:HD, :], rbc[:], MULT)

            # out projection: out[t, :] = ctx[t, :] @ WoT + bo
            for tt in range(TCH // 128):
                o_sb = opool.tile([128, D], F32, name="osb")
                for n0, n1 in ((0, 512), (512, 768)):
                    ps = mmp.tile([128, 512], F32, name="mmps")
                    for kt in range(NKT):
                        nc.tensor.matmul(
                            ps[:, :n1 - n0],
                            ctxT_sb[:, kt, tt * 128:(tt + 1) * 128],
                            w_sb["woT"][:, kt, n0:n1],
                            start=(kt == 0), stop=(kt == NKT - 1))
                        nc.vector.tensor_tensor(
                        o_sb[:, n0:n1], ps[:, :n1 - n0],
                        bo_sb[:, n0:n1], ADD)
                nc.sync.dma_start(out_d[ch * TCH + tt * 128:
                                        ch * TCH + (tt + 1) * 128, :], o_sb[:])

    nc.compile()
    return nc


def _get_nc():
    if "nc" not in _CACHE:
        _CACHE["nc"] = _build()
    return _CACHE["nc"]


def make_in_maps(x, Wq, bq, Wk, bk, Wv, bv, Wo, bo):
    wqT = np.ascontiguousarray(np.asarray(Wq, np.float32).T)
    wkT = np.ascontiguousarray(np.asarray(Wk, np.float32).T)
    wvT = np.ascontiguousarray(np.asarray(Wv, np.float32).T)
    woT = np.ascontiguousarray(np.asarray(Wo, np.float32).T)
    x = np.asarray(x, np.float32)
    in_maps = []
    for c in range(NCORE):
        b, half = divmod(c, 2)
        xT = np.ascontiguousarray(x[b, half * TOK:(half + 1) * TOK, :].T)
        xdT = np.ascontiguousarray(x[b, ::DIL, :].T)
        in_maps.append({
            "xT": xT, "xdT": xdT,
            "wqT": wqT, "wkT": wkT, "wvT": wvT, "woT": woT,
            "bq": np.asarray(bq, np.float32), "bk": np.asarray(bk, np.float32),
            "bv": np.asarray(bv, np.float32), "bo": np.asarray(bo, np.float32),
        })
    return in_maps


def assemble(results):
    out = np.empty((B, S, D), np.float32)
    for c in range(NCORE):
        b, half = divmod(c, 2)
        out[b, half * TOK:(half + 1) * TOK, :] = results[c]["out"]
    return out


def kernel(**inputs):
    nc = _get_nc()
    in_maps = make_in_maps(**inputs)
    res = run_bass_kernel_spmd(nc, in_maps, core_ids=list(range(NCORE)))
    return assemble(res.results)
